# revision 39
# baseline (speedup 1.0000x reference)
"""STBlock (temporal attn -> spatial attn -> ChebConv + residual, relu) on 8 trn2 cores.

Sharding: data-parallel over batch B=8, one batch element per core.

v2 design notes (vs 509us baseline): the baseline burned ~93us of PE on 256
tiny 24-col transposes and ~300us of Vector/Scalar on per-instruction copy
overhead. This version:
  - uploads x from host in all three layouts it is consumed in (natural
    f-major, transposed f-major, transposed t-major), killing the stage-3
    transpose storm entirely;
  - keeps a t-major (d' = t*32+f) column order for every intermediate, so the
    final Cheb+residual projection is 12 plain 128-contract matmuls with
    block weights and zero permutes/transposes at the tail;
  - computes each Chebyshev propagation directly in transposed form
    (Z1^T = sum_m Z0[m,d'] * L^T[m,n]), halving transpose passes;
  - applies E_att via a banded 768x768 block-diagonal matmul (15 tile pairs)
    built on-device from eatt with quadrant-safe small copies;
  - folds the bs-add into the S_pre PSUM accumulation via an identity matmul,
    and skips softmax max-subtraction (scores are provably < ~5);
  - batches every PSUM->SBUF drain to >=384-col single instructions and
    round-robins them across Vector/GpSimd/Scalar.

Layouts (per core):
  d  = f*24+t (f-major), d' = t*32+f (t-major); out^T row = t*64+g.
  All partition offsets must be 32-aligned (BIR verifier quadrant rule), so
  f-blocks (24 rows/cols) are placed at 32-strides with zero padding.
  XNP[8]  (128n, 1024)  bf16   natural padded (col 32f+t), for score_t
  XT96P[8](128d+,1024n) bf16   x^T f-major padded (row 32j+u per 4-f group)
  XTT[6]  (128d',1024n) bf16   x^T t-major, residual rhs in projection
  TT96[8] (96d, 1024n)  bf16   x_TA^T compact f-major (E-mult out)
  AN[8]   (128n, 768d') bf16   x_TA natural t-major (transpose+permute of TT)
  SG[8]   (128n, 1024m) bf16   sigmoid(score_s)
  SATB    (128m, 8x1024n) bf16 S_att^T, m-tile blocks along free dim
  Z0T/Z1T/Z2T[6] (128d', 1024n) bf16; Z0N/Z1N[8] (128n, 768d') bf16
  out^T   (1536, 1024) bf16 -> host transposes back
"""
import numpy as np

B, N, F, T, G = 8, 1024, 32, 24, 64
D = F * T            # 768
NCH = N // 128       # 8 n-chunks
DCH = D // 128       # 6 d-tiles
QO = 12              # out^T tiles (1536 rows)

_compiled = {}


def _build():
    if "nc" in _compiled:
        return _compiled["nc"]
    import concourse.mybir as mybir
    import concourse.bacc as bacc
    from concourse import tile

    FP = mybir.dt.float32
    BF = mybir.dt.bfloat16
    F8 = mybir.dt.float8e4
    AF = mybir.ActivationFunctionType
    OP = mybir.AluOpType
    DR = mybir.MatmulPerfMode.DoubleRow

    nc = bacc.Bacc("TRN2", target_bir_lowering=False, debug=False)

    xnp_d = nc.dram_tensor("xnp", (N, 1024), F8, kind="ExternalInput").ap()
    xt8f_d = nc.dram_tensor("xt8f", (D, N), F8, kind="ExternalInput").ap()
    bigi_d = nc.dram_tensor("bigi", (128, 384), BF, kind="ExternalInput").ap()
    xtt_d = nc.dram_tensor("xtt", (D, N), BF, kind="ExternalInput").ap()
    identb_d = nc.dram_tensor("identb", (128, 128), BF, kind="ExternalInput").ap()
    ident8_d = nc.dram_tensor("ident8", (128, 128), F8, kind="ExternalInput").ap()
    vetb_d = nc.dram_tensor("vetb", (T, T), BF, kind="ExternalInput").ap()
    be_d = nc.dram_tensor("be", (T, T), FP, kind="ExternalInput").ap()
    vst_d = nc.dram_tensor("vst8", (N, N), F8, kind="ExternalInput").ap()
    bst_d = nc.dram_tensor("bst", (N, N), BF, kind="ExternalInput").ap()
    lt8_d = nc.dram_tensor("lt8", (N, N), F8, kind="ExternalInput").ap()
    wpb_d = nc.dram_tensor("wpb", (128, QO * 4 * 128), BF, kind="ExternalInput").ap()
    bias_d = nc.dram_tensor("bias128", (128, 1), FP, kind="ExternalInput").ap()
    out_d = nc.dram_tensor("out", (QO * 128, N), BF, kind="ExternalOutput").ap()

    with tile.TileContext(nc) as tc:
        with (
            tc.tile_pool(name="persist", bufs=1) as pp,
            tc.tile_pool(name="stream", bufs=1) as sp,
            tc.tile_pool(name="psb", bufs=2, space="PSUM") as psb,
            tc.tile_pool(name="pst", bufs=3, space="PSUM") as pst,
            tc.tile_pool(name="ps1", bufs=1, space="PSUM") as ps1,
        ):
            # round-robin for copy/cast work across DVE / Pool engines
            # (Act is kept for activations + a share of copies where idle)
            _rr = [0]
            PSUM_SPACE = tile.bass.MemorySpace.PSUM

            def copy_rr(dst, src, engines=None):
                if engines is None:
                    # GpSimd cannot touch PSUM
                    if src.space == PSUM_SPACE or dst.space == PSUM_SPACE:
                        engines = (nc.vector, nc.scalar)
                    else:
                        engines = (nc.vector, nc.gpsimd)
                e = engines[_rr[0] % len(engines)]
                _rr[0] += 1
                if e is nc.scalar:
                    nc.scalar.activation(dst, src, AF.Copy)
                else:
                    e.tensor_copy(dst, src)

            def scaled_rr(dst, src, scale):
                if _rr[0] % 2 == 0:
                    nc.vector.tensor_scalar_mul(dst, src, scale)
                else:
                    nc.scalar.activation(dst, src, AF.Copy, scale=scale)
                _rr[0] += 1

            # ---- constants / inputs ----
            identb = pp.tile([128, 128], BF, tag="identb")
            nc.sync.dma_start(identb[:], identb_d[:])
            ident8 = pp.tile([128, 128], F8, tag="ident8")
            nc.sync.dma_start(ident8[:], ident8_d[:])
            vetb = pp.tile([T, T], BF, tag="vetb")
            nc.sync.dma_start(vetb[:], vetb_d[:])
            be = pp.tile([T, T], FP, tag="be")
            nc.sync.dma_start(be[:], be_d[:])
            # preload Act function tables off the critical path
            warm = sp.tile([1, 1], FP, tag="warm")
            nc.scalar.activation(warm[:], identb[0:1, 0:1], AF.Sigmoid)
            nc.scalar.activation(warm[:], identb[0:1, 0:1], AF.Exp)
            nc.scalar.activation(warm[:], identb[0:1, 0:1], AF.Relu)

            XNP = []
            for i in range(NCH):
                t_ = pp.tile([128, 1024], F8, name=f"xnpA{i}", tag=f"A{i}")
                nc.sync.dma_start(t_[:], xnp_d[i * 128:(i + 1) * 128, :])
                XNP.append(t_)
            # x^T f-major fp8, one tile: col block p = d-tile p (DR pairing)
            xt8f = pp.tile([128, DCH * N], F8, tag="xt8f")
            for p in range(DCH):
                nc.sync.dma_start(xt8f[:, p * N:(p + 1) * N],
                                  xt8f_d[p * 128:(p + 1) * 128, :])
            xt8fv = xt8f[:].rearrange("q (p n) -> q p n", p=DCH)
            bigi = pp.tile([128, 384], BF, tag="bigi")
            nc.sync.dma_start(bigi[:], bigi_d[:])

            # ---- S1: score_t = sum_{n,f} x[n,f,t] x[n,f,u] ----
            # XNP col blocks of 128 = 4 f's at 32-stride padding; the Gram of
            # each block has the per-f 24x24 diagonal blocks at 32-aligned
            # partition offsets. Garbage off-diagonal blocks are ignored.
            acc128 = pp.tile([128, 128], FP, tag="acc128")
            for g2 in range(8):
                pt = ps1.tile([128, 128], FP, tag="st")
                for i in range(NCH):
                    sl = XNP[i][:, g2 * 128:(g2 + 1) * 128]
                    nc.tensor.matmul(pt[:], sl, sl,
                                     start=(i == 0), stop=(i == NCH - 1))
                if g2 == 0:
                    nc.vector.tensor_copy(acc128[:], pt[:])
                else:
                    nc.vector.tensor_tensor(acc128[:], acc128[:], pt[:], op=OP.add)
            # TensorTensor needs equal base partitions for SBUF inputs, so
            # first move the three off-base diagonal blocks down to base 0.
            dg = []
            for j, eng in ((1, nc.vector), (2, nc.gpsimd), (3, nc.vector)):
                t_ = sp.tile([T, T], FP, name=f"dg{j}", tag=f"dg{j}")
                eng.tensor_copy(t_[:], acc128[32 * j:32 * j + 24,
                                              32 * j:32 * j + 24])
                dg.append(t_)
            sct_a = sp.tile([T, T], FP, tag="sct_a")
            nc.vector.tensor_tensor(sct_a[:], acc128[0:24, 0:24],
                                    dg[0][:], op=OP.add)
            sct_b = sp.tile([T, T], FP, tag="sct_b")
            nc.gpsimd.tensor_tensor(sct_b[:], dg[1][:], dg[2][:], op=OP.add)
            score_t = sp.tile([T, T], FP, tag="score_t")
            nc.vector.tensor_tensor(score_t[:], sct_a[:], sct_b[:], op=OP.add)

            # ---- S2: E_att = softmax(Ve @ sigmoid(score_t) + be) ----
            sigb = sp.tile([T, T], BF, tag="sigb")
            nc.scalar.activation(sigb[:], score_t[:], AF.Sigmoid)
            ps_e = ps1.tile([T, T], FP, tag="st")
            nc.tensor.matmul(ps_e[:], vetb[:], sigb[:], start=True, stop=True)
            epre = sp.tile([T, T], FP, tag="epre")
            nc.vector.tensor_tensor(epre[:], ps_e[:], be[:], op=OP.add)
            eexp = sp.tile([T, T], FP, tag="eexp")
            esum = sp.tile([T, 1], FP, tag="esum")
            nc.scalar.activation(eexp[:], epre[:], AF.Exp, accum_out=esum[:])
            einv = sp.tile([T, 1], FP, tag="einv")
            nc.vector.reciprocal(einv[:], esum[:])
            eatt = sp.tile([T, T], BF, tag="eatt")
            nc.vector.tensor_scalar_mul(eatt[:], eexp[:], einv[:])

            # EBIG: banded blocks of blockdiag(E_att x32), built on the PE
            # with shift-matrix (identity-slice) matmuls, then cast to fp8.
            bands = []
            for p in range(DCH):
                qs = []
                for q in (p - 1, p, p + 1):
                    if not 0 <= q < DCH:
                        continue
                    fs = [f for f in range(F)
                          if 24 * f < 128 * q + 128 and 24 * f + 24 > 128 * q
                          and 24 * f < 128 * p + 128 and 24 * f + 24 > 128 * p]
                    if fs:
                        qs.append((q, fs))
                bands.append(qs)
            soff = {}
            s = 0
            for p in range(DCH):
                for q, _ in bands[p]:
                    soff[(p, q)] = s
                    s += 1
            NB = s  # 14 blocks
            e4r = pp.tile([128, T], BF, tag="e4r")
            nc.gpsimd.memset(e4r[:], 0.0)
            nc.vector.tensor_copy(e4r[0:24, :], eatt[:])
            ebig = pp.tile([128, NB * 128], F8, tag="ebig")
            nc.gpsimd.memset(ebig[:], 0.0)
            for half in range(2):
                blo = half * 7
                bhi = min(NB, blo + 7)
                pe_b = psb.tile([128, N], FP, tag="big")
                ranges = {}
                for p in range(DCH):
                    for q, fs in bands[p]:
                        sb = soff[(p, q)]
                        if not blo <= sb < bhi:
                            continue
                        for f in fs:
                            dlt = 24 * f - 128 * q
                            c0 = 24 * f - 128 * p
                            t0, t1 = max(0, -c0), min(24, 128 - c0)
                            cc = (sb - blo) * 128 + c0 + t0
                            nc.tensor.matmul(
                                pe_b[:, cc:cc + (t1 - t0)],
                                bigi[:, 128 - dlt:256 - dlt],
                                e4r[:, t0:t1], start=True, stop=True)
                            lo, hi = ranges.get(sb, (10 ** 9, -1))
                            ranges[sb] = (min(lo, c0 + t0), max(hi, c0 + t1))
                for sb, (lo, hi) in sorted(ranges.items()):
                    copy_rr(ebig[:, sb * 128 + lo:sb * 128 + hi],
                            pe_b[:, (sb - blo) * 128 + lo:(sb - blo) * 128 + hi])

            # ---- S3: TT8 = x_TA^T (f-major) via banded fp8 matmul ----
            tt8 = pp.tile([128, DCH * N], F8, tag="tt8")
            for p in range(DCH):
                pb = psb.tile([128, N], FP, tag="big")
                qs = bands[p]
                q0 = qs[0][0]
                s0 = soff[(p, q0)]
                for h in range(2):
                    nc.tensor.matmul(
                        pb[:, h * 512:(h + 1) * 512],
                        ebig[:, s0 * 128:(s0 + 2) * 128].rearrange(
                            "q (k c) -> q k c", k=2),
                        xt8fv[:, q0:q0 + 2, h * 512:(h + 1) * 512],
                        start=True, stop=(len(qs) == 2), perf_mode=DR)
                    if len(qs) == 3:
                        q2 = qs[2][0]
                        s2 = soff[(p, q2)]
                        nc.tensor.matmul(
                            pb[:, h * 512:(h + 1) * 512],
                            ebig[:, s2 * 128:(s2 + 1) * 128],
                            xt8f[:, q2 * N + h * 512:q2 * N + (h + 1) * 512],
                            start=False, stop=True)
                copy_rr(tt8[:, p * N:(p + 1) * N], pb[:])
            tt8v = tt8[:].rearrange("q (p n) -> q p n", p=DCH)

            # ---- S5 (score_s -> SG) interleaved with S4 (AN build) ----
            sg8 = pp.tile([128, NCH * N], F8, tag="sg8")
            sg8v = sg8[:].rearrange("q (m n) -> q m n", m=NCH)
            # x_TA natural, t-major cols, fp8 (m-blocks along free dim)
            anb = pp.tile([128, NCH * D], F8, tag="anb")
            anbv = anb[:].rearrange("q (m d) -> q m d", m=NCH)
            for i in range(NCH):
                pb = psb.tile([128, N], FP, tag="big")
                for h in range(2):
                    for a2 in range(3):  # DoubleRow over d-tile pairs
                        nc.tensor.matmul(
                            pb[:, h * 512:(h + 1) * 512],
                            tt8v[:, 2 * a2:2 * a2 + 2, i * 128:(i + 1) * 128],
                            tt8v[:, 2 * a2:2 * a2 + 2, h * 512:(h + 1) * 512],
                            start=(a2 == 0), stop=(a2 == 2), perf_mode=DR)
                nc.scalar.activation(sg8[:, i * N:(i + 1) * N], pb[:], AF.Sigmoid)

                # fp8 transpose must write psum with element step 2
                pa = pst.tile([128, 2 * D], F8, tag="tr")
                pav = pa[:].rearrange("q (c two) -> q two c", two=2)
                for p in range(DCH):
                    nc.tensor.transpose(pav[:, 0, p * 128:(p + 1) * 128],
                                        tt8[:, p * N + i * 128:p * N + (i + 1) * 128],
                                        ident8[:])
                # permute f-major step-2 psum -> t-major fp8; contiguous
                # 32B writes per t; halves run on DVE and Act in parallel
                dstv = anb[:, i * D:(i + 1) * D].rearrange(
                    "q (t f) -> q t f", t=T, f=F)
                srcv = pa[:].rearrange("q (f t two) -> q t f two",
                                       f=F, t=T, two=2)
                nc.vector.tensor_copy(dstv[:, 0:12, :].unsqueeze(3),
                                      srcv[:, 0:12, :, 0:1])
                nc.scalar.activation(dstv[:, 12:24, :].unsqueeze(3),
                                     srcv[:, 12:24, :, 0:1], AF.Copy)

            # Vs^T as one (128, 8x1024) fp8 tile: col block m holds m-tile rows
            vst8 = pp.tile([128, NCH * N], F8, tag="vst8")
            for m in range(NCH):
                nc.sync.dma_start(vst8[:, m * N:(m + 1) * N],
                                  vst_d[m * 128:(m + 1) * 128, :])
            vst8v = vst8[:].rearrange("q (m n) -> q m n", m=NCH)

            # ---- S6: S_att rows + softmax; S7: transpose into SATB ----
            satb = pp.tile([128, NCH * N], F8, tag="satb")
            satbv = satb[:].rearrange("q (m n) -> q m n", m=NCH)
            for i in range(NCH):
                bsb = sp.tile([128, N], BF, tag="bsb", bufs=2)
                nc.sync.dma_start(bsb[:], bst_d[i * 128:(i + 1) * 128, :])
                pb = psb.tile([128, N], FP, tag="big")
                for h in range(2):
                    for a2 in range(4):  # DoubleRow over m-tile pairs
                        nc.tensor.matmul(
                            pb[:, h * 512:(h + 1) * 512],
                            vst8v[:, 2 * a2:2 * a2 + 2, i * 128:(i + 1) * 128],
                            sg8v[:, 2 * a2:2 * a2 + 2, h * 512:(h + 1) * 512],
                            start=(a2 == 0), stop=False, perf_mode=DR)
                    nc.tensor.matmul(
                        pb[:, h * 512:(h + 1) * 512],
                        identb[:], bsb[:, h * 512:(h + 1) * 512],
                        start=False, stop=True)
                sexp = sp.tile([128, N], FP, tag="sexp", bufs=2)
                ssum = sp.tile([128, 1], FP, tag="ssum", bufs=2)
                nc.scalar.activation(sexp[:], pb[:], AF.Exp, accum_out=ssum[:])
                sinv = sp.tile([128, 1], FP, tag="sinv", bufs=2)
                nc.vector.reciprocal(sinv[:], ssum[:])
                # x512 keeps softmax weights above the fp8e4m3 subnormal floor;
                # the Z0T drain divides it back out.
                sa = sp.tile([128, N], BF, tag="sa", bufs=2)
                nc.gpsimd.tensor_scalar(sa[:], sexp[:], sinv[:], 512.0,
                                        op0=OP.mult, op1=OP.mult)
                for grp in range(2):
                    pq = pst.tile([128, 512], BF, tag="tr")
                    for k in range(4):
                        p = grp * 4 + k
                        nc.tensor.transpose(pq[:, k * 128:(k + 1) * 128],
                                            sa[:, p * 128:(p + 1) * 128],
                                            identb[:])
                    dst = satb[:].rearrange("q (p n) -> q p n", p=NCH)[
                        :, grp * 4:grp * 4 + 4, i * 128:(i + 1) * 128]
                    copy_rr(dst, pq[:].rearrange("q (p n) -> q p n", p=4))

            # late DMAs: not needed before S8+, keep head bandwidth clear
            wpb = pp.tile([128, QO * 4 * 128], BF, tag="wpb")
            nc.sync.dma_start(wpb[:], wpb_d[:])
            bias128 = pp.tile([128, 1], FP, tag="bias128")
            nc.sync.dma_start(bias128[:], bias_d[:])
            XTT = []
            for p in range(DCH):
                t_ = pp.tile([128, N], BF, name=f"xttX{p}", tag=f"X{p}")
                nc.sync.dma_start(t_[:], xtt_d[p * 128:(p + 1) * 128, :])
                XTT.append(t_)
            # 8*L^T as one (128, 8x1024) fp8 tile (m-tile blocks on cols)
            lt8 = pp.tile([128, NCH * N], F8, tag="lt8")
            for m in range(NCH):
                nc.sync.dma_start(lt8[:, m * N:(m + 1) * N],
                                  lt8_d[m * 128:(m + 1) * 128, :])
            lt8v = lt8[:].rearrange("q (m n) -> q m n", m=NCH)

            # ---- S8: Z0T = (S_att @ x_TA)^T directly (t-major) ----
            Z0T = []
            for p in range(DCH):
                pb = psb.tile([128, N], FP, tag="big")
                for h in range(2):
                    for a2 in range(4):  # DoubleRow over m-tile pairs
                        nc.tensor.matmul(
                            pb[:, h * 512:(h + 1) * 512],
                            anbv[:, 2 * a2:2 * a2 + 2, p * 128:(p + 1) * 128],
                            satbv[:, 2 * a2:2 * a2 + 2, h * 512:(h + 1) * 512],
                            start=(a2 == 0), stop=(a2 == 3), perf_mode=DR)
                t_ = pp.tile([128, N], BF, name=f"z0tT{p}", tag=f"T{p}")
                if p % 2 == 0:
                    nc.vector.tensor_scalar_mul(t_[:], pb[:], 1.0 / 512.0)
                else:
                    nc.scalar.activation(t_[:], pb[:], AF.Copy, scale=1.0 / 512.0)
                Z0T.append(t_)

            # ---- S9: Z0N = 64*transpose(Z0T), fp8 (m-blocks on cols) ----
            z0nb = pp.tile([128, NCH * D], F8, tag="z0nb")
            z0nbv = z0nb[:].rearrange("q (m d) -> q m d", m=NCH)
            for i in range(NCH):
                for grp in range(2):
                    pz = pst.tile([128, 384], BF, tag="tr")
                    for k in range(3):
                        p = grp * 3 + k
                        nc.tensor.transpose(pz[:, k * 128:(k + 1) * 128],
                                            Z0T[p][:, i * 128:(i + 1) * 128],
                                            identb[:])
                    scaled_rr(z0nb[:, i * D + grp * 384:i * D + (grp + 1) * 384],
                              pz[:], 64.0)

            # ---- S10: Z1T[d',n] = sum_m Z0[m,d'] L^T[m,n] = (L@Z0)^T ----
            Z1T = []
            for p in range(DCH):
                pb = psb.tile([128, N], FP, tag="big")
                for h in range(2):
                    for a2 in range(4):
                        nc.tensor.matmul(
                            pb[:, h * 512:(h + 1) * 512],
                            z0nbv[:, 2 * a2:2 * a2 + 2, p * 128:(p + 1) * 128],
                            lt8v[:, 2 * a2:2 * a2 + 2, h * 512:(h + 1) * 512],
                            start=(a2 == 0), stop=(a2 == 3), perf_mode=DR)
                t_ = pp.tile([128, N], BF, name=f"z1tV{p}", tag=f"V{p}")
                scaled_rr(t_[:], pb[:], 1.0 / 512.0)
                Z1T.append(t_)

            # ---- S11: Z1N = 64*transpose(Z1T), fp8 ----
            z1nb = pp.tile([128, NCH * D], F8, tag="z1nb")
            z1nbv = z1nb[:].rearrange("q (m d) -> q m d", m=NCH)
            for i in range(NCH):
                for grp in range(2):
                    pz = pst.tile([128, 384], BF, tag="tr")
                    for k in range(3):
                        p = grp * 3 + k
                        nc.tensor.transpose(pz[:, k * 128:(k + 1) * 128],
                                            Z1T[p][:, i * 128:(i + 1) * 128],
                                            identb[:])
                    scaled_rr(z1nb[:, i * D + grp * 384:i * D + (grp + 1) * 384],
                              pz[:], 64.0)

            # ---- S12: Z2T = 2*(L@Z1)^T - Z0T ----
            Z2T = []
            for p in range(DCH):
                pb = psb.tile([128, N], FP, tag="big")
                for h in range(2):
                    for a2 in range(4):
                        nc.tensor.matmul(
                            pb[:, h * 512:(h + 1) * 512],
                            z1nbv[:, 2 * a2:2 * a2 + 2, p * 128:(p + 1) * 128],
                            lt8v[:, 2 * a2:2 * a2 + 2, h * 512:(h + 1) * 512],
                            start=(a2 == 0), stop=(a2 == 3), perf_mode=DR)
                # psum holds 512*(L@Z1); Z2 = psum/256 - Z0
                zc = sp.tile([128, N], BF, tag="z2c", bufs=2)
                scaled_rr(zc[:], pb[:], 1.0 / 256.0)
                t_ = pp.tile([128, N], BF, name=f"z2tZ{p}", tag=f"Z2{p}")
                if p % 2 == 0:
                    nc.vector.tensor_tensor(t_[:], zc[:], Z0T[p][:], op=OP.subtract)
                else:
                    nc.gpsimd.tensor_tensor(t_[:], zc[:], Z0T[p][:], op=OP.subtract)
                Z2T.append(t_)

            # ---- S13: projection (Cheb k=0..2 + residual), bias, relu ----
            for q in range(QO):
                p = q // 2
                pb = psb.tile([128, N], FP, tag="big")
                rhs4 = (Z0T[p], Z1T[p], Z2T[p], XTT[p])
                for h in range(2):
                    for k in range(4):
                        nc.tensor.matmul(
                            pb[:, h * 512:(h + 1) * 512],
                            wpb[:, (4 * q + k) * 128:(4 * q + k + 1) * 128],
                            rhs4[k][:, h * 512:(h + 1) * 512],
                            start=(k == 0), stop=(k == 3))
                ob = sp.tile([128, N], BF, tag="outbuf", bufs=2)
                if q % 2 == 0:
                    nc.scalar.activation(ob[:], pb[:], AF.Relu, bias=bias128[:])
                else:
                    nc.vector.tensor_scalar(ob[:], pb[:], bias128[:], 0.0,
                                            op0=OP.add, op1=OP.max)
                nc.sync.dma_start(out_d[q * 128:(q + 1) * 128, :], ob[:])

    nc.compile()
    _compiled["nc"] = nc
    return nc


def _host_prep(x, edge_index, edge_weight, Ve, be, Vs, bs, cheb_W, cheb_b, res_W, res_b):
    import ml_dtypes
    BF = ml_dtypes.bfloat16
    row = np.asarray(edge_index[0]).astype(np.int64)
    col = np.asarray(edge_index[1]).astype(np.int64)
    w = np.asarray(edge_weight, np.float64).copy()
    w[row == col] = 0.0
    deg = np.zeros(N, np.float64)
    np.add.at(deg, row, w)
    dis = np.where(deg > 0, 1.0 / np.sqrt(np.where(deg > 0, deg, 1.0)), 0.0)
    norm = -dis[row] * w * dis[col]
    L = np.zeros((N, N), np.float64)
    np.add.at(L, (col, row), norm)
    LT = np.ascontiguousarray(L.T.astype(np.float32))

    cheb_W = np.asarray(cheb_W, np.float32)
    res_W = np.asarray(res_W, np.float32)
    # wpb[p, (4q+k)*128 + c] = blk(q,k)[p, c]; out^T tile q rows (t,g) with
    # t = 2q + c//64, contracting t-major tile p=q//2 rows (t', f)
    wq = np.zeros((QO, 4, 128, 128), np.float32)
    Wlist = [cheb_W[0], cheb_W[1], cheb_W[2], res_W.T]  # each (F, G)
    for q in range(QO):
        off = 0 if q % 2 == 0 else 2
        for b_ in range(2):
            a = b_ + off
            for k in range(4):
                wq[q, k, 32 * a:32 * a + 32, 64 * b_:64 * b_ + 64] = Wlist[k]
    wpb = np.ascontiguousarray(
        wq.transpose(2, 0, 1, 3).reshape(128, QO * 4 * 128)).astype(BF)

    b64 = (np.asarray(cheb_b, np.float32) + np.asarray(res_b, np.float32))
    bias128 = np.concatenate([b64, b64]).reshape(128, 1).astype(np.float32)

    import ml_dtypes as mld
    bigi = np.zeros((128, 384), np.float32)
    bigi[np.arange(128), 128 + np.arange(128)] = 1.0
    return {
        "bigi": bigi.astype(mld.bfloat16),
        "identb": np.eye(128, dtype=np.float32).astype(BF),
        "ident8": np.eye(128, dtype=np.float32).astype(mld.float8_e4m3),
        "vetb": np.ascontiguousarray(np.asarray(Ve, np.float32).T).astype(BF),
        "be": np.ascontiguousarray(np.asarray(be, np.float32)[0]),
        "vst8": np.ascontiguousarray(
            np.asarray(Vs, np.float32).T).astype(mld.float8_e4m3),
        "bst": np.ascontiguousarray(np.asarray(bs, np.float32)[0]).astype(BF),
        "lt8": (8.0 * LT).astype(mld.float8_e4m3),
        "wpb": wpb,
        "bias128": bias128,
    }


TRACE = False
LAST = {}


def kernel(x, edge_index, edge_weight, Ve, be, Vs, bs, cheb_W, cheb_b, res_W, res_b):
    from concourse.bass_utils import run_bass_kernel_spmd
    import ml_dtypes
    BF = ml_dtypes.bfloat16
    F8H = ml_dtypes.float8_e4m3

    x = np.asarray(x, np.float32)
    shared = _host_prep(x, edge_index, edge_weight, Ve, be, Vs, bs,
                        cheb_W, cheb_b, res_W, res_b)
    nc = _build()
    in_maps = []
    for b in range(B):
        m = dict(shared)
        xb = x[b]                                   # (N, F, T)
        xnp = np.zeros((N, F, 32), np.float32)      # col 32f+t, zero padded
        xnp[:, :, :T] = xb
        m["xnp"] = np.ascontiguousarray(xnp.reshape(N, 1024)).astype(F8H)
        xtf = xb.reshape(N, D).T                    # (768, N), d = f*24+t
        m["xt8f"] = np.ascontiguousarray(xtf).astype(F8H)
        m["xtt"] = np.ascontiguousarray(
            xb.transpose(2, 1, 0).reshape(D, N)).astype(BF)  # d' = t*32+f
        in_maps.append(m)
    res = run_bass_kernel_spmd(nc, in_maps, list(range(B)), trace=TRACE)
    LAST["res"] = res
    out = np.stack(
        [r["out"].astype(np.float32).reshape(T, G, N).transpose(2, 1, 0)
         for r in res.results], axis=0)
    return out


# revision 40
# speedup vs baseline: 1.0006x; 1.0006x over previous
"""STBlock (temporal attn -> spatial attn -> ChebConv + residual, relu) on 8 trn2 cores.

Sharding: data-parallel over batch B=8, one batch element per core.

Design (509us baseline -> ~152us): the baseline burned ~93us of PE on 256
tiny 24-col transposes and ~300us of Vector/Scalar on per-instruction copy
overhead. This version:
  - uploads x from host in the three layouts it is consumed in (padded
    natural fp8, transposed f-major fp8, transposed t-major bf16), killing
    the transpose storm entirely;
  - keeps a t-major (d' = t*32+f) column order for every intermediate, so the
    final Cheb+residual projection is 12 plain 128-contract matmuls with
    block weights and zero permutes at the tail; out is written transposed
    (t*64+g rows) and the host untransposes it;
  - computes each Chebyshev propagation directly in transposed form
    (Z1^T[d',n] = sum_m Z0[m,d'] L^T[m,n]), halving transpose passes;
  - applies E_att via a banded 768x768 block-diag(E x32) matmul whose 14
    banded 128x128 blocks are built ON the PE from eatt with shift-matrix
    (identity-slice) matmuls - no misaligned-partition copies;
  - runs the five big N-contractions (score_s, S_pre, Z0, L@Z0, L@Z1) in
    fp8e4m3 with DoubleRow perf mode (K=256/instr). fp8 underflow is the
    trap: softmax weights (~1e-3) and Z values (~1e-2) sit below the e4m3
    subnormal floor, so S_att is scaled x512 and Z^T x64 into fp8, with the
    scales divided back out in the PSUM drains (free in copy/Act scale);
  - folds the bs-add into the S_pre PSUM accumulation via an identity
    matmul, skips softmax max-subtraction (|logits| < ~5 provably), and
    spreads drains/softmax across Vector/Act/Pool (Pool cannot touch PSUM).

Layouts: d = f*24+t (f-major), d' = t*32+f (t-major); out^T row = t*64+g.
Partition offsets must be 32-aligned (BIR quadrant rule) and TensorTensor
needs equal SBUF base partitions, so f-blocks (24 wide) are padded to
32-strides (XNP cols, score_t Gram diagonals). fp8 PE transposes must write
PSUM at element step 2; the AN drain compacts + permutes f->t-major in two
halved copies (contiguous 32B writes) run on DVE and Act in parallel.
"""
import numpy as np

B, N, F, T, G = 8, 1024, 32, 24, 64
D = F * T            # 768
NCH = N // 128       # 8 n-chunks
DCH = D // 128       # 6 d-tiles
QO = 12              # out^T tiles (1536 rows)

_compiled = {}


def _build():
    if "nc" in _compiled:
        return _compiled["nc"]
    import concourse.mybir as mybir
    import concourse.bacc as bacc
    from concourse import tile

    FP = mybir.dt.float32
    BF = mybir.dt.bfloat16
    F8 = mybir.dt.float8e4
    AF = mybir.ActivationFunctionType
    OP = mybir.AluOpType
    DR = mybir.MatmulPerfMode.DoubleRow

    nc = bacc.Bacc("TRN2", target_bir_lowering=False, debug=False)

    xnp_d = nc.dram_tensor("xnp", (N, 1024), F8, kind="ExternalInput").ap()
    xt8f_d = nc.dram_tensor("xt8f", (D, N), F8, kind="ExternalInput").ap()
    bigi_d = nc.dram_tensor("bigi", (128, 384), BF, kind="ExternalInput").ap()
    xtt_d = nc.dram_tensor("xtt", (D, N), BF, kind="ExternalInput").ap()
    identb_d = nc.dram_tensor("identb", (128, 128), BF, kind="ExternalInput").ap()
    ident8_d = nc.dram_tensor("ident8", (128, 128), F8, kind="ExternalInput").ap()
    vetb_d = nc.dram_tensor("vetb", (T, T), BF, kind="ExternalInput").ap()
    be_d = nc.dram_tensor("be", (T, T), FP, kind="ExternalInput").ap()
    vst_d = nc.dram_tensor("vst8", (N, N), F8, kind="ExternalInput").ap()
    bst_d = nc.dram_tensor("bst", (N, N), BF, kind="ExternalInput").ap()
    lt8_d = nc.dram_tensor("lt8", (N, N), F8, kind="ExternalInput").ap()
    wpb_d = nc.dram_tensor("wpb", (128, QO * 4 * 128), BF, kind="ExternalInput").ap()
    bias_d = nc.dram_tensor("bias128", (128, 1), FP, kind="ExternalInput").ap()
    out_d = nc.dram_tensor("out", (QO * 128, N), BF, kind="ExternalOutput").ap()

    with tile.TileContext(nc) as tc:
        with (
            tc.tile_pool(name="persist", bufs=1) as pp,
            tc.tile_pool(name="stream", bufs=1) as sp,
            tc.tile_pool(name="psb", bufs=2, space="PSUM") as psb,
            tc.tile_pool(name="pst", bufs=3, space="PSUM") as pst,
            tc.tile_pool(name="ps1", bufs=1, space="PSUM") as ps1,
        ):
            # round-robin for copy/cast work across DVE / Pool engines
            # (Act is kept for activations + a share of copies where idle)
            _rr = [0]
            PSUM_SPACE = tile.bass.MemorySpace.PSUM

            def copy_rr(dst, src, engines=None):
                if engines is None:
                    # GpSimd cannot touch PSUM
                    if src.space == PSUM_SPACE or dst.space == PSUM_SPACE:
                        engines = (nc.vector, nc.scalar)
                    else:
                        engines = (nc.vector, nc.gpsimd)
                e = engines[_rr[0] % len(engines)]
                _rr[0] += 1
                if e is nc.scalar:
                    nc.scalar.activation(dst, src, AF.Copy)
                else:
                    e.tensor_copy(dst, src)

            def scaled_rr(dst, src, scale):
                if _rr[0] % 2 == 0:
                    nc.vector.tensor_scalar_mul(dst, src, scale)
                else:
                    nc.scalar.activation(dst, src, AF.Copy, scale=scale)
                _rr[0] += 1

            # ---- constants / inputs ----
            identb = pp.tile([128, 128], BF, tag="identb")
            nc.sync.dma_start(identb[:], identb_d[:])
            ident8 = pp.tile([128, 128], F8, tag="ident8")
            nc.sync.dma_start(ident8[:], ident8_d[:])
            vetb = pp.tile([T, T], BF, tag="vetb")
            nc.sync.dma_start(vetb[:], vetb_d[:])
            be = pp.tile([T, T], FP, tag="be")
            nc.sync.dma_start(be[:], be_d[:])
            # preload Act function tables off the critical path
            warm = sp.tile([1, 1], FP, tag="warm")
            nc.scalar.activation(warm[:], identb[0:1, 0:1], AF.Sigmoid)
            nc.scalar.activation(warm[:], identb[0:1, 0:1], AF.Exp)
            nc.scalar.activation(warm[:], identb[0:1, 0:1], AF.Relu)

            XNP = []
            for i in range(NCH):
                t_ = pp.tile([128, 1024], F8, name=f"xnpA{i}", tag=f"A{i}")
                nc.sync.dma_start(t_[:], xnp_d[i * 128:(i + 1) * 128, :])
                XNP.append(t_)
            # x^T f-major fp8, one tile: col block p = d-tile p (DR pairing)
            xt8f = pp.tile([128, DCH * N], F8, tag="xt8f")
            for p in range(DCH):
                nc.sync.dma_start(xt8f[:, p * N:(p + 1) * N],
                                  xt8f_d[p * 128:(p + 1) * 128, :])
            xt8fv = xt8f[:].rearrange("q (p n) -> q p n", p=DCH)
            bigi = pp.tile([128, 384], BF, tag="bigi")
            nc.sync.dma_start(bigi[:], bigi_d[:])

            # ---- S1: score_t = sum_{n,f} x[n,f,t] x[n,f,u] ----
            # XNP col blocks of 128 = 4 f's at 32-stride padding; the Gram of
            # each block has the per-f 24x24 diagonal blocks at 32-aligned
            # partition offsets. Garbage off-diagonal blocks are ignored.
            acc128 = pp.tile([128, 128], FP, tag="acc128")
            for g2 in range(8):
                pt = ps1.tile([128, 128], FP, tag="st")
                for i in range(NCH):
                    sl = XNP[i][:, g2 * 128:(g2 + 1) * 128]
                    nc.tensor.matmul(pt[:], sl, sl,
                                     start=(i == 0), stop=(i == NCH - 1))
                if g2 == 0:
                    nc.vector.tensor_copy(acc128[:], pt[:])
                else:
                    nc.vector.tensor_tensor(acc128[:], acc128[:], pt[:], op=OP.add)
            # TensorTensor needs equal base partitions for SBUF inputs, so
            # first move the three off-base diagonal blocks down to base 0.
            dg = []
            for j, eng in ((1, nc.vector), (2, nc.gpsimd), (3, nc.vector)):
                t_ = sp.tile([T, T], FP, name=f"dg{j}", tag=f"dg{j}")
                eng.tensor_copy(t_[:], acc128[32 * j:32 * j + 24,
                                              32 * j:32 * j + 24])
                dg.append(t_)
            sct_a = sp.tile([T, T], FP, tag="sct_a")
            nc.vector.tensor_tensor(sct_a[:], acc128[0:24, 0:24],
                                    dg[0][:], op=OP.add)
            sct_b = sp.tile([T, T], FP, tag="sct_b")
            nc.gpsimd.tensor_tensor(sct_b[:], dg[1][:], dg[2][:], op=OP.add)
            score_t = sp.tile([T, T], FP, tag="score_t")
            nc.vector.tensor_tensor(score_t[:], sct_a[:], sct_b[:], op=OP.add)

            # ---- S2: E_att = softmax(Ve @ sigmoid(score_t) + be) ----
            sigb = sp.tile([T, T], BF, tag="sigb")
            nc.scalar.activation(sigb[:], score_t[:], AF.Sigmoid)
            ps_e = ps1.tile([T, T], FP, tag="st")
            nc.tensor.matmul(ps_e[:], vetb[:], sigb[:], start=True, stop=True)
            epre = sp.tile([T, T], FP, tag="epre")
            nc.vector.tensor_tensor(epre[:], ps_e[:], be[:], op=OP.add)
            eexp = sp.tile([T, T], FP, tag="eexp")
            esum = sp.tile([T, 1], FP, tag="esum")
            nc.scalar.activation(eexp[:], epre[:], AF.Exp, accum_out=esum[:])
            einv = sp.tile([T, 1], FP, tag="einv")
            nc.vector.reciprocal(einv[:], esum[:])
            eatt = sp.tile([T, T], BF, tag="eatt")
            nc.vector.tensor_scalar_mul(eatt[:], eexp[:], einv[:])

            # EBIG: banded blocks of blockdiag(E_att x32), built on the PE
            # with shift-matrix (identity-slice) matmuls, then cast to fp8.
            bands = []
            for p in range(DCH):
                qs = []
                for q in (p - 1, p, p + 1):
                    if not 0 <= q < DCH:
                        continue
                    fs = [f for f in range(F)
                          if 24 * f < 128 * q + 128 and 24 * f + 24 > 128 * q
                          and 24 * f < 128 * p + 128 and 24 * f + 24 > 128 * p]
                    if fs:
                        qs.append((q, fs))
                bands.append(qs)
            soff = {}
            s = 0
            for p in range(DCH):
                for q, _ in bands[p]:
                    soff[(p, q)] = s
                    s += 1
            NB = s  # 14 blocks
            e4r = pp.tile([128, T], BF, tag="e4r")
            nc.gpsimd.memset(e4r[:], 0.0)
            nc.vector.tensor_copy(e4r[0:24, :], eatt[:])
            ebig = pp.tile([128, NB * 128], F8, tag="ebig")
            nc.gpsimd.memset(ebig[:], 0.0)
            for half in range(2):
                blo = half * 7
                bhi = min(NB, blo + 7)
                pe_b = psb.tile([128, N], FP, tag="big")
                ranges = {}
                for p in range(DCH):
                    for q, fs in bands[p]:
                        sb = soff[(p, q)]
                        if not blo <= sb < bhi:
                            continue
                        for f in fs:
                            dlt = 24 * f - 128 * q
                            c0 = 24 * f - 128 * p
                            t0, t1 = max(0, -c0), min(24, 128 - c0)
                            cc = (sb - blo) * 128 + c0 + t0
                            nc.tensor.matmul(
                                pe_b[:, cc:cc + (t1 - t0)],
                                bigi[:, 128 - dlt:256 - dlt],
                                e4r[:, t0:t1], start=True, stop=True)
                            lo, hi = ranges.get(sb, (10 ** 9, -1))
                            ranges[sb] = (min(lo, c0 + t0), max(hi, c0 + t1))
                for sb, (lo, hi) in sorted(ranges.items()):
                    copy_rr(ebig[:, sb * 128 + lo:sb * 128 + hi],
                            pe_b[:, (sb - blo) * 128 + lo:(sb - blo) * 128 + hi])

            # ---- S3: TT8 = x_TA^T (f-major) via banded fp8 matmul ----
            tt8 = pp.tile([128, DCH * N], F8, tag="tt8")
            for p in range(DCH):
                pb = psb.tile([128, N], FP, tag="big")
                qs = bands[p]
                q0 = qs[0][0]
                s0 = soff[(p, q0)]
                for h in range(2):
                    nc.tensor.matmul(
                        pb[:, h * 512:(h + 1) * 512],
                        ebig[:, s0 * 128:(s0 + 2) * 128].rearrange(
                            "q (k c) -> q k c", k=2),
                        xt8fv[:, q0:q0 + 2, h * 512:(h + 1) * 512],
                        start=True, stop=(len(qs) == 2), perf_mode=DR)
                    if len(qs) == 3:
                        q2 = qs[2][0]
                        s2 = soff[(p, q2)]
                        nc.tensor.matmul(
                            pb[:, h * 512:(h + 1) * 512],
                            ebig[:, s2 * 128:(s2 + 1) * 128],
                            xt8f[:, q2 * N + h * 512:q2 * N + (h + 1) * 512],
                            start=False, stop=True)
                copy_rr(tt8[:, p * N:(p + 1) * N], pb[:])
            tt8v = tt8[:].rearrange("q (p n) -> q p n", p=DCH)

            # ---- S5 (score_s -> SG) interleaved with S4 (AN build) ----
            sg8 = pp.tile([128, NCH * N], F8, tag="sg8")
            sg8v = sg8[:].rearrange("q (m n) -> q m n", m=NCH)
            # x_TA natural, t-major cols, fp8 (m-blocks along free dim)
            anb = pp.tile([128, NCH * D], F8, tag="anb")
            anbv = anb[:].rearrange("q (m d) -> q m d", m=NCH)
            for i in range(NCH):
                pb = psb.tile([128, N], FP, tag="big")
                for h in range(2):
                    for a2 in range(3):  # DoubleRow over d-tile pairs
                        nc.tensor.matmul(
                            pb[:, h * 512:(h + 1) * 512],
                            tt8v[:, 2 * a2:2 * a2 + 2, i * 128:(i + 1) * 128],
                            tt8v[:, 2 * a2:2 * a2 + 2, h * 512:(h + 1) * 512],
                            start=(a2 == 0), stop=(a2 == 2), perf_mode=DR)
                nc.scalar.activation(sg8[:, i * N:(i + 1) * N], pb[:], AF.Sigmoid)

                # fp8 transpose must write psum with element step 2
                pa = pst.tile([128, 2 * D], F8, tag="tr")
                pav = pa[:].rearrange("q (c two) -> q two c", two=2)
                for p in range(DCH):
                    nc.tensor.transpose(pav[:, 0, p * 128:(p + 1) * 128],
                                        tt8[:, p * N + i * 128:p * N + (i + 1) * 128],
                                        ident8[:])
                # permute f-major step-2 psum -> t-major fp8; contiguous
                # 32B writes per t; halves run on DVE and Act in parallel
                dstv = anb[:, i * D:(i + 1) * D].rearrange(
                    "q (t f) -> q t f", t=T, f=F)
                srcv = pa[:].rearrange("q (f t two) -> q t f two",
                                       f=F, t=T, two=2)
                nc.vector.tensor_copy(dstv[:, 0:12, :].unsqueeze(3),
                                      srcv[:, 0:12, :, 0:1])
                nc.scalar.activation(dstv[:, 12:24, :].unsqueeze(3),
                                     srcv[:, 12:24, :, 0:1], AF.Copy)

            # Vs^T as one (128, 8x1024) fp8 tile: col block m holds m-tile rows
            vst8 = pp.tile([128, NCH * N], F8, tag="vst8")
            for m in range(NCH):
                nc.sync.dma_start(vst8[:, m * N:(m + 1) * N],
                                  vst_d[m * 128:(m + 1) * 128, :])
            vst8v = vst8[:].rearrange("q (m n) -> q m n", m=NCH)

            # ---- S6: S_att rows + softmax; S7: transpose into SATB ----
            satb = pp.tile([128, NCH * N], F8, tag="satb")
            satbv = satb[:].rearrange("q (m n) -> q m n", m=NCH)
            for i in range(NCH):
                bsb = sp.tile([128, N], BF, tag="bsb", bufs=2)
                nc.sync.dma_start(bsb[:], bst_d[i * 128:(i + 1) * 128, :])
                pb = psb.tile([128, N], FP, tag="big")
                for h in range(2):
                    for a2 in range(4):  # DoubleRow over m-tile pairs
                        nc.tensor.matmul(
                            pb[:, h * 512:(h + 1) * 512],
                            vst8v[:, 2 * a2:2 * a2 + 2, i * 128:(i + 1) * 128],
                            sg8v[:, 2 * a2:2 * a2 + 2, h * 512:(h + 1) * 512],
                            start=(a2 == 0), stop=False, perf_mode=DR)
                    nc.tensor.matmul(
                        pb[:, h * 512:(h + 1) * 512],
                        identb[:], bsb[:, h * 512:(h + 1) * 512],
                        start=False, stop=True)
                sexp = sp.tile([128, N], FP, tag="sexp", bufs=2)
                ssum = sp.tile([128, 1], FP, tag="ssum", bufs=2)
                nc.scalar.activation(sexp[:], pb[:], AF.Exp, accum_out=ssum[:])
                sinv = sp.tile([128, 1], FP, tag="sinv", bufs=2)
                nc.vector.reciprocal(sinv[:], ssum[:])
                # x512 keeps softmax weights above the fp8e4m3 subnormal floor;
                # the Z0T drain divides it back out.
                sa = sp.tile([128, N], BF, tag="sa", bufs=2)
                nc.gpsimd.tensor_scalar(sa[:], sexp[:], sinv[:], 512.0,
                                        op0=OP.mult, op1=OP.mult)
                for grp in range(2):
                    pq = pst.tile([128, 512], BF, tag="tr")
                    for k in range(4):
                        p = grp * 4 + k
                        nc.tensor.transpose(pq[:, k * 128:(k + 1) * 128],
                                            sa[:, p * 128:(p + 1) * 128],
                                            identb[:])
                    dst = satb[:].rearrange("q (p n) -> q p n", p=NCH)[
                        :, grp * 4:grp * 4 + 4, i * 128:(i + 1) * 128]
                    copy_rr(dst, pq[:].rearrange("q (p n) -> q p n", p=4))

            # late DMAs: not needed before S8+, keep head bandwidth clear
            wpb = pp.tile([128, QO * 4 * 128], BF, tag="wpb")
            nc.sync.dma_start(wpb[:], wpb_d[:])
            bias128 = pp.tile([128, 1], FP, tag="bias128")
            nc.sync.dma_start(bias128[:], bias_d[:])
            XTT = []
            for p in range(DCH):
                t_ = pp.tile([128, N], BF, name=f"xttX{p}", tag=f"X{p}")
                nc.sync.dma_start(t_[:], xtt_d[p * 128:(p + 1) * 128, :])
                XTT.append(t_)
            # 8*L^T as one (128, 8x1024) fp8 tile (m-tile blocks on cols)
            lt8 = pp.tile([128, NCH * N], F8, tag="lt8")
            for m in range(NCH):
                nc.sync.dma_start(lt8[:, m * N:(m + 1) * N],
                                  lt8_d[m * 128:(m + 1) * 128, :])
            lt8v = lt8[:].rearrange("q (m n) -> q m n", m=NCH)

            # ---- S8: Z0T = (S_att @ x_TA)^T directly (t-major) ----
            Z0T = []
            for p in range(DCH):
                pb = psb.tile([128, N], FP, tag="big")
                for h in range(2):
                    for a2 in range(4):  # DoubleRow over m-tile pairs
                        nc.tensor.matmul(
                            pb[:, h * 512:(h + 1) * 512],
                            anbv[:, 2 * a2:2 * a2 + 2, p * 128:(p + 1) * 128],
                            satbv[:, 2 * a2:2 * a2 + 2, h * 512:(h + 1) * 512],
                            start=(a2 == 0), stop=(a2 == 3), perf_mode=DR)
                t_ = pp.tile([128, N], BF, name=f"z0tT{p}", tag=f"T{p}")
                if p % 2 == 0:
                    nc.vector.tensor_scalar_mul(t_[:], pb[:], 1.0 / 512.0)
                else:
                    nc.scalar.activation(t_[:], pb[:], AF.Copy, scale=1.0 / 512.0)
                Z0T.append(t_)

            # ---- S9: Z0N = 64*transpose(Z0T), fp8 (m-blocks on cols) ----
            z0nb = pp.tile([128, NCH * D], F8, tag="z0nb")
            z0nbv = z0nb[:].rearrange("q (m d) -> q m d", m=NCH)
            for i in range(NCH):
                for grp in range(2):
                    pz = pst.tile([128, 384], BF, tag="tr")
                    for k in range(3):
                        p = grp * 3 + k
                        nc.tensor.transpose(pz[:, k * 128:(k + 1) * 128],
                                            Z0T[p][:, i * 128:(i + 1) * 128],
                                            identb[:])
                    scaled_rr(z0nb[:, i * D + grp * 384:i * D + (grp + 1) * 384],
                              pz[:], 64.0)

            # ---- S10: Z1T[d',n] = sum_m Z0[m,d'] L^T[m,n] = (L@Z0)^T ----
            Z1T = []
            for p in range(DCH):
                pb = psb.tile([128, N], FP, tag="big")
                for h in range(2):
                    for a2 in range(4):
                        nc.tensor.matmul(
                            pb[:, h * 512:(h + 1) * 512],
                            z0nbv[:, 2 * a2:2 * a2 + 2, p * 128:(p + 1) * 128],
                            lt8v[:, 2 * a2:2 * a2 + 2, h * 512:(h + 1) * 512],
                            start=(a2 == 0), stop=(a2 == 3), perf_mode=DR)
                t_ = pp.tile([128, N], BF, name=f"z1tV{p}", tag=f"V{p}")
                scaled_rr(t_[:], pb[:], 1.0 / 512.0)
                Z1T.append(t_)

            # ---- S11: Z1N = 64*transpose(Z1T), fp8 ----
            z1nb = pp.tile([128, NCH * D], F8, tag="z1nb")
            z1nbv = z1nb[:].rearrange("q (m d) -> q m d", m=NCH)
            for i in range(NCH):
                for grp in range(2):
                    pz = pst.tile([128, 384], BF, tag="tr")
                    for k in range(3):
                        p = grp * 3 + k
                        nc.tensor.transpose(pz[:, k * 128:(k + 1) * 128],
                                            Z1T[p][:, i * 128:(i + 1) * 128],
                                            identb[:])
                    scaled_rr(z1nb[:, i * D + grp * 384:i * D + (grp + 1) * 384],
                              pz[:], 64.0)

            # ---- S12: Z2T = 2*(L@Z1)^T - Z0T ----
            Z2T = []
            for p in range(DCH):
                pb = psb.tile([128, N], FP, tag="big")
                for h in range(2):
                    for a2 in range(4):
                        nc.tensor.matmul(
                            pb[:, h * 512:(h + 1) * 512],
                            z1nbv[:, 2 * a2:2 * a2 + 2, p * 128:(p + 1) * 128],
                            lt8v[:, 2 * a2:2 * a2 + 2, h * 512:(h + 1) * 512],
                            start=(a2 == 0), stop=(a2 == 3), perf_mode=DR)
                # psum holds 512*(L@Z1); Z2 = psum/256 - Z0
                zc = sp.tile([128, N], BF, tag="z2c", bufs=2)
                scaled_rr(zc[:], pb[:], 1.0 / 256.0)
                t_ = pp.tile([128, N], BF, name=f"z2tZ{p}", tag=f"Z2{p}")
                if p % 2 == 0:
                    nc.vector.tensor_tensor(t_[:], zc[:], Z0T[p][:], op=OP.subtract)
                else:
                    nc.gpsimd.tensor_tensor(t_[:], zc[:], Z0T[p][:], op=OP.subtract)
                Z2T.append(t_)

            # ---- S13: projection (Cheb k=0..2 + residual), bias, relu ----
            for q in range(QO):
                p = q // 2
                pb = psb.tile([128, N], FP, tag="big")
                rhs4 = (Z0T[p], Z1T[p], Z2T[p], XTT[p])
                for h in range(2):
                    for k in range(4):
                        nc.tensor.matmul(
                            pb[:, h * 512:(h + 1) * 512],
                            wpb[:, (4 * q + k) * 128:(4 * q + k + 1) * 128],
                            rhs4[k][:, h * 512:(h + 1) * 512],
                            start=(k == 0), stop=(k == 3))
                ob = sp.tile([128, N], BF, tag="outbuf", bufs=2)
                if q % 2 == 0:
                    nc.scalar.activation(ob[:], pb[:], AF.Relu, bias=bias128[:])
                else:
                    nc.vector.tensor_scalar(ob[:], pb[:], bias128[:], 0.0,
                                            op0=OP.add, op1=OP.max)
                nc.sync.dma_start(out_d[q * 128:(q + 1) * 128, :], ob[:])

    nc.compile()
    _compiled["nc"] = nc
    return nc


def _host_prep(x, edge_index, edge_weight, Ve, be, Vs, bs, cheb_W, cheb_b, res_W, res_b):
    import ml_dtypes
    BF = ml_dtypes.bfloat16
    row = np.asarray(edge_index[0]).astype(np.int64)
    col = np.asarray(edge_index[1]).astype(np.int64)
    w = np.asarray(edge_weight, np.float64).copy()
    w[row == col] = 0.0
    deg = np.zeros(N, np.float64)
    np.add.at(deg, row, w)
    dis = np.where(deg > 0, 1.0 / np.sqrt(np.where(deg > 0, deg, 1.0)), 0.0)
    norm = -dis[row] * w * dis[col]
    L = np.zeros((N, N), np.float64)
    np.add.at(L, (col, row), norm)
    LT = np.ascontiguousarray(L.T.astype(np.float32))

    cheb_W = np.asarray(cheb_W, np.float32)
    res_W = np.asarray(res_W, np.float32)
    # wpb[p, (4q+k)*128 + c] = blk(q,k)[p, c]; out^T tile q rows (t,g) with
    # t = 2q + c//64, contracting t-major tile p=q//2 rows (t', f)
    wq = np.zeros((QO, 4, 128, 128), np.float32)
    Wlist = [cheb_W[0], cheb_W[1], cheb_W[2], res_W.T]  # each (F, G)
    for q in range(QO):
        off = 0 if q % 2 == 0 else 2
        for b_ in range(2):
            a = b_ + off
            for k in range(4):
                wq[q, k, 32 * a:32 * a + 32, 64 * b_:64 * b_ + 64] = Wlist[k]
    wpb = np.ascontiguousarray(
        wq.transpose(2, 0, 1, 3).reshape(128, QO * 4 * 128)).astype(BF)

    b64 = (np.asarray(cheb_b, np.float32) + np.asarray(res_b, np.float32))
    bias128 = np.concatenate([b64, b64]).reshape(128, 1).astype(np.float32)

    import ml_dtypes as mld
    bigi = np.zeros((128, 384), np.float32)
    bigi[np.arange(128), 128 + np.arange(128)] = 1.0
    return {
        "bigi": bigi.astype(mld.bfloat16),
        "identb": np.eye(128, dtype=np.float32).astype(BF),
        "ident8": np.eye(128, dtype=np.float32).astype(mld.float8_e4m3),
        "vetb": np.ascontiguousarray(np.asarray(Ve, np.float32).T).astype(BF),
        "be": np.ascontiguousarray(np.asarray(be, np.float32)[0]),
        "vst8": np.ascontiguousarray(
            np.asarray(Vs, np.float32).T).astype(mld.float8_e4m3),
        "bst": np.ascontiguousarray(np.asarray(bs, np.float32)[0]).astype(BF),
        "lt8": (8.0 * LT).astype(mld.float8_e4m3),
        "wpb": wpb,
        "bias128": bias128,
    }


TRACE = False
LAST = {}


def kernel(x, edge_index, edge_weight, Ve, be, Vs, bs, cheb_W, cheb_b, res_W, res_b):
    from concourse.bass_utils import run_bass_kernel_spmd
    import ml_dtypes
    BF = ml_dtypes.bfloat16
    F8H = ml_dtypes.float8_e4m3

    x = np.asarray(x, np.float32)
    shared = _host_prep(x, edge_index, edge_weight, Ve, be, Vs, bs,
                        cheb_W, cheb_b, res_W, res_b)
    nc = _build()
    in_maps = []
    for b in range(B):
        m = dict(shared)
        xb = x[b]                                   # (N, F, T)
        xnp = np.zeros((N, F, 32), np.float32)      # col 32f+t, zero padded
        xnp[:, :, :T] = xb
        m["xnp"] = np.ascontiguousarray(xnp.reshape(N, 1024)).astype(F8H)
        xtf = xb.reshape(N, D).T                    # (768, N), d = f*24+t
        m["xt8f"] = np.ascontiguousarray(xtf).astype(F8H)
        m["xtt"] = np.ascontiguousarray(
            xb.transpose(2, 1, 0).reshape(D, N)).astype(BF)  # d' = t*32+f
        in_maps.append(m)
    res = run_bass_kernel_spmd(nc, in_maps, list(range(B)), trace=TRACE)
    LAST["res"] = res
    out = np.stack(
        [r["out"].astype(np.float32).reshape(T, G, N).transpose(2, 1, 0)
         for r in res.results], axis=0)
    return out


# revision 41
# speedup vs baseline: 1.0175x; 1.0169x over previous
"""STBlock (temporal attn -> spatial attn -> ChebConv + residual, relu) on 8 trn2 cores.

Sharding: data-parallel over batch B=8, one batch element per core.

Design (509us baseline -> ~152us): the baseline burned ~93us of PE on 256
tiny 24-col transposes and ~300us of Vector/Scalar on per-instruction copy
overhead. This version:
  - uploads x from host in the three layouts it is consumed in (padded
    natural fp8, transposed f-major fp8, transposed t-major bf16), killing
    the transpose storm entirely;
  - keeps a t-major (d' = t*32+f) column order for every intermediate, so the
    final Cheb+residual projection is 12 plain 128-contract matmuls with
    block weights and zero permutes at the tail; out is written transposed
    (t*64+g rows) and the host untransposes it;
  - computes each Chebyshev propagation directly in transposed form
    (Z1^T[d',n] = sum_m Z0[m,d'] L^T[m,n]), halving transpose passes;
  - applies E_att via a banded 768x768 block-diag(E x32) matmul whose 14
    banded 128x128 blocks are built ON the PE from eatt with shift-matrix
    (identity-slice) matmuls - no misaligned-partition copies;
  - runs the five big N-contractions (score_s, S_pre, Z0, L@Z0, L@Z1) in
    fp8e4m3 with DoubleRow perf mode (K=256/instr). fp8 underflow is the
    trap: softmax weights (~1e-3) and Z values (~1e-2) sit below the e4m3
    subnormal floor, so S_att is scaled x512 and Z^T x64 into fp8, with the
    scales divided back out in the PSUM drains (free in copy/Act scale);
  - folds the bs-add into the S_pre PSUM accumulation via an identity
    matmul, skips softmax max-subtraction (|logits| < ~5 provably), and
    spreads drains/softmax across Vector/Act/Pool (Pool cannot touch PSUM).

Layouts: d = f*24+t (f-major), d' = t*32+f (t-major); out^T row = t*64+g.
Partition offsets must be 32-aligned (BIR quadrant rule) and TensorTensor
needs equal SBUF base partitions, so f-blocks (24 wide) are padded to
32-strides (XNP cols, score_t Gram diagonals). fp8 PE transposes must write
PSUM at element step 2; the AN drain compacts + permutes f->t-major in two
halved copies (contiguous 32B writes) run on DVE and Act in parallel.
"""
import numpy as np

B, N, F, T, G = 8, 1024, 32, 24, 64
D = F * T            # 768
NCH = N // 128       # 8 n-chunks
DCH = D // 128       # 6 d-tiles
QO = 12              # out^T tiles (1536 rows)

_compiled = {}


def _build():
    if "nc" in _compiled:
        return _compiled["nc"]
    import concourse.mybir as mybir
    import concourse.bacc as bacc
    from concourse import tile

    FP = mybir.dt.float32
    BF = mybir.dt.bfloat16
    F8 = mybir.dt.float8e4
    AF = mybir.ActivationFunctionType
    OP = mybir.AluOpType
    DR = mybir.MatmulPerfMode.DoubleRow

    nc = bacc.Bacc("TRN2", target_bir_lowering=False, debug=False)

    xnp_d = nc.dram_tensor("xnp", (N, 1024), F8, kind="ExternalInput").ap()
    xt8f_d = nc.dram_tensor("xt8f", (D, N), F8, kind="ExternalInput").ap()
    bigi_d = nc.dram_tensor("bigi", (128, 384), BF, kind="ExternalInput").ap()
    xtt_d = nc.dram_tensor("xtt", (D, N), BF, kind="ExternalInput").ap()
    identb_d = nc.dram_tensor("identb", (128, 128), BF, kind="ExternalInput").ap()
    ident8_d = nc.dram_tensor("ident8", (128, 128), F8, kind="ExternalInput").ap()
    vetb_d = nc.dram_tensor("vetb", (T, T), BF, kind="ExternalInput").ap()
    be_d = nc.dram_tensor("be", (T, T), FP, kind="ExternalInput").ap()
    vst_d = nc.dram_tensor("vst8", (N, N), F8, kind="ExternalInput").ap()
    bst_d = nc.dram_tensor("bst", (N, N), BF, kind="ExternalInput").ap()
    lt8_d = nc.dram_tensor("lt8", (N, N), F8, kind="ExternalInput").ap()
    wpb_d = nc.dram_tensor("wpb", (128, QO * 4 * 128), BF, kind="ExternalInput").ap()
    bias_d = nc.dram_tensor("bias128", (128, 1), FP, kind="ExternalInput").ap()
    out_d = nc.dram_tensor("out", (QO * 128, N), BF, kind="ExternalOutput").ap()

    with tile.TileContext(nc) as tc:
        with (
            tc.tile_pool(name="persist", bufs=1) as pp,
            tc.tile_pool(name="stream", bufs=1) as sp,
            tc.tile_pool(name="psb", bufs=2, space="PSUM") as psb,
            tc.tile_pool(name="pst", bufs=3, space="PSUM") as pst,
            tc.tile_pool(name="ps1", bufs=1, space="PSUM") as ps1,
        ):
            # round-robin for copy/cast work across DVE / Pool engines
            # (Act is kept for activations + a share of copies where idle)
            _rr = [0]
            PSUM_SPACE = tile.bass.MemorySpace.PSUM

            def copy_rr(dst, src, engines=None):
                if engines is None:
                    # GpSimd cannot touch PSUM
                    if src.space == PSUM_SPACE or dst.space == PSUM_SPACE:
                        engines = (nc.vector, nc.scalar)
                    else:
                        engines = (nc.vector, nc.gpsimd)
                e = engines[_rr[0] % len(engines)]
                _rr[0] += 1
                if e is nc.scalar:
                    nc.scalar.activation(dst, src, AF.Copy)
                else:
                    e.tensor_copy(dst, src)

            def scaled_rr(dst, src, scale):
                if _rr[0] % 2 == 0:
                    nc.vector.tensor_scalar_mul(dst, src, scale)
                else:
                    nc.scalar.activation(dst, src, AF.Copy, scale=scale)
                _rr[0] += 1

            # ---- constants / inputs ----
            identb = pp.tile([128, 128], BF, tag="identb")
            nc.sync.dma_start(identb[:], identb_d[:])
            ident8 = pp.tile([128, 128], F8, tag="ident8")
            nc.sync.dma_start(ident8[:], ident8_d[:])
            vetb = pp.tile([T, T], BF, tag="vetb")
            nc.sync.dma_start(vetb[:], vetb_d[:])
            be = pp.tile([T, T], FP, tag="be")
            nc.sync.dma_start(be[:], be_d[:])

            xnpb = pp.tile([128, NCH * 1024], F8, tag="xnpb")
            for i in range(NCH):
                nc.sync.dma_start(xnpb[:, i * 1024:(i + 1) * 1024],
                                  xnp_d[i * 128:(i + 1) * 128, :])
            xnpbv = xnpb[:].rearrange("q (m c) -> q m c", m=NCH)
            # x^T f-major fp8, one tile: col block p = d-tile p (DR pairing)
            xt8f = pp.tile([128, DCH * N], F8, tag="xt8f")
            for p in range(DCH):
                nc.sync.dma_start(xt8f[:, p * N:(p + 1) * N],
                                  xt8f_d[p * 128:(p + 1) * 128, :])
            xt8fv = xt8f[:].rearrange("q (p n) -> q p n", p=DCH)
            bigi = pp.tile([128, 384], BF, tag="bigi")
            nc.sync.dma_start(bigi[:], bigi_d[:])

            # ---- S1: score_t = sum_{n,f} x[n,f,t] x[n,f,u] ----
            # XNP col blocks of 128 = 4 f's at 32-stride padding; the Gram of
            # each block has the per-f 24x24 diagonal blocks at 32-aligned
            # partition offsets. Garbage off-diagonal blocks are ignored.
            acc128 = pp.tile([128, 128], FP, tag="acc128")
            for g2 in range(8):
                pt = ps1.tile([128, 128], FP, tag="st")
                for a2 in range(4):  # DoubleRow over n-chunk pairs
                    sl = xnpbv[:, 2 * a2:2 * a2 + 2,
                               g2 * 128:(g2 + 1) * 128]
                    nc.tensor.matmul(pt[:], sl, sl,
                                     start=(a2 == 0), stop=(a2 == 3),
                                     perf_mode=DR)
                if g2 == 0:
                    nc.vector.tensor_copy(acc128[:], pt[:])
                else:
                    nc.vector.tensor_tensor(acc128[:], acc128[:], pt[:], op=OP.add)
            # TensorTensor needs equal base partitions for SBUF inputs, so
            # first move the three off-base diagonal blocks down to base 0.
            dg = []
            for j, eng in ((1, nc.vector), (2, nc.gpsimd), (3, nc.vector)):
                t_ = sp.tile([T, T], FP, name=f"dg{j}", tag=f"dg{j}")
                eng.tensor_copy(t_[:], acc128[32 * j:32 * j + 24,
                                              32 * j:32 * j + 24])
                dg.append(t_)
            sct_a = sp.tile([T, T], FP, tag="sct_a")
            nc.vector.tensor_tensor(sct_a[:], acc128[0:24, 0:24],
                                    dg[0][:], op=OP.add)
            sct_b = sp.tile([T, T], FP, tag="sct_b")
            nc.gpsimd.tensor_tensor(sct_b[:], dg[1][:], dg[2][:], op=OP.add)
            score_t = sp.tile([T, T], FP, tag="score_t")
            nc.vector.tensor_tensor(score_t[:], sct_a[:], sct_b[:], op=OP.add)

            # ---- S2: E_att = softmax(Ve @ sigmoid(score_t) + be) ----
            sigb = sp.tile([T, T], BF, tag="sigb")
            nc.scalar.activation(sigb[:], score_t[:], AF.Sigmoid)
            ps_e = ps1.tile([T, T], FP, tag="st")
            nc.tensor.matmul(ps_e[:], vetb[:], sigb[:], start=True, stop=True)
            epre = sp.tile([T, T], FP, tag="epre")
            nc.vector.tensor_tensor(epre[:], ps_e[:], be[:], op=OP.add)
            eexp = sp.tile([T, T], FP, tag="eexp")
            esum = sp.tile([T, 1], FP, tag="esum")
            nc.scalar.activation(eexp[:], epre[:], AF.Exp, accum_out=esum[:])
            einv = sp.tile([T, 1], FP, tag="einv")
            nc.vector.reciprocal(einv[:], esum[:])
            eatt = sp.tile([T, T], BF, tag="eatt")
            nc.vector.tensor_scalar_mul(eatt[:], eexp[:], einv[:])

            # EBIG: banded blocks of blockdiag(E_att x32), built on the PE
            # with shift-matrix (identity-slice) matmuls, then cast to fp8.
            bands = []
            for p in range(DCH):
                qs = []
                for q in (p - 1, p, p + 1):
                    if not 0 <= q < DCH:
                        continue
                    fs = [f for f in range(F)
                          if 24 * f < 128 * q + 128 and 24 * f + 24 > 128 * q
                          and 24 * f < 128 * p + 128 and 24 * f + 24 > 128 * p]
                    if fs:
                        qs.append((q, fs))
                bands.append(qs)
            soff = {}
            s = 0
            for p in range(DCH):
                for q, _ in bands[p]:
                    soff[(p, q)] = s
                    s += 1
            NB = s  # 14 blocks
            e4r = pp.tile([128, T], BF, tag="e4r")
            nc.gpsimd.memset(e4r[:], 0.0)
            nc.vector.tensor_copy(e4r[0:24, :], eatt[:])
            ebig = pp.tile([128, NB * 128], F8, tag="ebig")
            nc.gpsimd.memset(ebig[:], 0.0)
            for half in range(2):
                blo = half * 7
                bhi = min(NB, blo + 7)
                pe_b = psb.tile([128, N], FP, tag="big")
                ranges = {}
                for p in range(DCH):
                    for q, fs in bands[p]:
                        sb = soff[(p, q)]
                        if not blo <= sb < bhi:
                            continue
                        for f in fs:
                            dlt = 24 * f - 128 * q
                            c0 = 24 * f - 128 * p
                            t0, t1 = max(0, -c0), min(24, 128 - c0)
                            cc = (sb - blo) * 128 + c0 + t0
                            nc.tensor.matmul(
                                pe_b[:, cc:cc + (t1 - t0)],
                                bigi[:, 128 - dlt:256 - dlt],
                                e4r[:, t0:t1], start=True, stop=True)
                            lo, hi = ranges.get(sb, (10 ** 9, -1))
                            ranges[sb] = (min(lo, c0 + t0), max(hi, c0 + t1))
                for sb, (lo, hi) in sorted(ranges.items()):
                    copy_rr(ebig[:, sb * 128 + lo:sb * 128 + hi],
                            pe_b[:, (sb - blo) * 128 + lo:(sb - blo) * 128 + hi])

            # ---- S3: TT8 = x_TA^T (f-major) via banded fp8 matmul ----
            tt8 = pp.tile([128, DCH * N], F8, tag="tt8")
            for p in range(DCH):
                pb = psb.tile([128, N], FP, tag="big")
                qs = bands[p]
                q0 = qs[0][0]
                s0 = soff[(p, q0)]
                for h in range(2):
                    nc.tensor.matmul(
                        pb[:, h * 512:(h + 1) * 512],
                        ebig[:, s0 * 128:(s0 + 2) * 128].rearrange(
                            "q (k c) -> q k c", k=2),
                        xt8fv[:, q0:q0 + 2, h * 512:(h + 1) * 512],
                        start=True, stop=(len(qs) == 2), perf_mode=DR)
                    if len(qs) == 3:
                        q2 = qs[2][0]
                        s2 = soff[(p, q2)]
                        nc.tensor.matmul(
                            pb[:, h * 512:(h + 1) * 512],
                            ebig[:, s2 * 128:(s2 + 1) * 128],
                            xt8f[:, q2 * N + h * 512:q2 * N + (h + 1) * 512],
                            start=False, stop=True)
                copy_rr(tt8[:, p * N:(p + 1) * N], pb[:])
            tt8v = tt8[:].rearrange("q (p n) -> q p n", p=DCH)

            # ---- S5 (score_s -> SG) interleaved with S4 (AN build) ----
            sg8 = pp.tile([128, NCH * N], F8, tag="sg8")
            sg8v = sg8[:].rearrange("q (m n) -> q m n", m=NCH)
            # x_TA natural, t-major cols, fp8 (m-blocks along free dim)
            anb = pp.tile([128, NCH * D], F8, tag="anb")
            anbv = anb[:].rearrange("q (m d) -> q m d", m=NCH)
            for i in range(NCH):
                pb = psb.tile([128, N], FP, tag="big")
                for h in range(2):
                    for a2 in range(3):  # DoubleRow over d-tile pairs
                        nc.tensor.matmul(
                            pb[:, h * 512:(h + 1) * 512],
                            tt8v[:, 2 * a2:2 * a2 + 2, i * 128:(i + 1) * 128],
                            tt8v[:, 2 * a2:2 * a2 + 2, h * 512:(h + 1) * 512],
                            start=(a2 == 0), stop=(a2 == 2), perf_mode=DR)
                nc.scalar.activation(sg8[:, i * N:(i + 1) * N], pb[:], AF.Sigmoid)

                # fp8 transpose must write psum with element step 2
                pa = pst.tile([128, 2 * D], F8, tag="tr")
                pav = pa[:].rearrange("q (c two) -> q two c", two=2)
                for p in range(DCH):
                    nc.tensor.transpose(pav[:, 0, p * 128:(p + 1) * 128],
                                        tt8[:, p * N + i * 128:p * N + (i + 1) * 128],
                                        ident8[:])
                # permute f-major step-2 psum -> t-major fp8; contiguous
                # 32B writes per t; halves run on DVE and Act in parallel
                dstv = anb[:, i * D:(i + 1) * D].rearrange(
                    "q (t f) -> q t f", t=T, f=F)
                srcv = pa[:].rearrange("q (f t two) -> q t f two",
                                       f=F, t=T, two=2)
                nc.vector.tensor_copy(dstv[:, 0:12, :].unsqueeze(3),
                                      srcv[:, 0:12, :, 0:1])
                nc.scalar.activation(dstv[:, 12:24, :].unsqueeze(3),
                                     srcv[:, 12:24, :, 0:1], AF.Copy)

            # Vs^T as one (128, 8x1024) fp8 tile: col block m holds m-tile rows
            vst8 = pp.tile([128, NCH * N], F8, tag="vst8")
            for m in range(NCH):
                nc.sync.dma_start(vst8[:, m * N:(m + 1) * N],
                                  vst_d[m * 128:(m + 1) * 128, :])
            vst8v = vst8[:].rearrange("q (m n) -> q m n", m=NCH)

            # ---- S6: S_att rows + softmax; S7: transpose into SATB ----
            satb = pp.tile([128, NCH * N], F8, tag="satb")
            satbv = satb[:].rearrange("q (m n) -> q m n", m=NCH)
            for i in range(NCH):
                bsb = sp.tile([128, N], BF, tag="bsb", bufs=2)
                nc.sync.dma_start(bsb[:], bst_d[i * 128:(i + 1) * 128, :])
                pb = psb.tile([128, N], FP, tag="big")
                for h in range(2):
                    for a2 in range(4):  # DoubleRow over m-tile pairs
                        nc.tensor.matmul(
                            pb[:, h * 512:(h + 1) * 512],
                            vst8v[:, 2 * a2:2 * a2 + 2, i * 128:(i + 1) * 128],
                            sg8v[:, 2 * a2:2 * a2 + 2, h * 512:(h + 1) * 512],
                            start=(a2 == 0), stop=False, perf_mode=DR)
                    nc.tensor.matmul(
                        pb[:, h * 512:(h + 1) * 512],
                        identb[:], bsb[:, h * 512:(h + 1) * 512],
                        start=False, stop=True)
                sexp = sp.tile([128, N], FP, tag="sexp", bufs=2)
                ssum = sp.tile([128, 1], FP, tag="ssum", bufs=2)
                nc.scalar.activation(sexp[:], pb[:], AF.Exp, accum_out=ssum[:])
                sinv = sp.tile([128, 1], FP, tag="sinv", bufs=2)
                nc.vector.reciprocal(sinv[:], ssum[:])
                # x512 keeps softmax weights above the fp8e4m3 subnormal floor;
                # the Z0T drain divides it back out.
                sa = sp.tile([128, N], BF, tag="sa", bufs=2)
                nc.gpsimd.tensor_scalar(sa[:], sexp[:], sinv[:], 512.0,
                                        op0=OP.mult, op1=OP.mult)
                for grp in range(2):
                    pq = pst.tile([128, 512], BF, tag="tr")
                    for k in range(4):
                        p = grp * 4 + k
                        nc.tensor.transpose(pq[:, k * 128:(k + 1) * 128],
                                            sa[:, p * 128:(p + 1) * 128],
                                            identb[:])
                    dst = satb[:].rearrange("q (p n) -> q p n", p=NCH)[
                        :, grp * 4:grp * 4 + 4, i * 128:(i + 1) * 128]
                    copy_rr(dst, pq[:].rearrange("q (p n) -> q p n", p=4))

            # late DMAs: not needed before S8+, keep head bandwidth clear
            wpb = pp.tile([128, QO * 4 * 128], BF, tag="wpb")
            nc.sync.dma_start(wpb[:], wpb_d[:])
            bias128 = pp.tile([128, 1], FP, tag="bias128")
            nc.sync.dma_start(bias128[:], bias_d[:])
            XTT = []
            for p in range(DCH):
                t_ = pp.tile([128, N], BF, name=f"xttX{p}", tag=f"X{p}")
                nc.sync.dma_start(t_[:], xtt_d[p * 128:(p + 1) * 128, :])
                XTT.append(t_)
            # 8*L^T as one (128, 8x1024) fp8 tile (m-tile blocks on cols)
            lt8 = pp.tile([128, NCH * N], F8, tag="lt8")
            for m in range(NCH):
                nc.sync.dma_start(lt8[:, m * N:(m + 1) * N],
                                  lt8_d[m * 128:(m + 1) * 128, :])
            lt8v = lt8[:].rearrange("q (m n) -> q m n", m=NCH)

            # ---- S8: Z0T = (S_att @ x_TA)^T directly (t-major) ----
            Z0T = []
            for p in range(DCH):
                pb = psb.tile([128, N], FP, tag="big")
                for h in range(2):
                    for a2 in range(4):  # DoubleRow over m-tile pairs
                        nc.tensor.matmul(
                            pb[:, h * 512:(h + 1) * 512],
                            anbv[:, 2 * a2:2 * a2 + 2, p * 128:(p + 1) * 128],
                            satbv[:, 2 * a2:2 * a2 + 2, h * 512:(h + 1) * 512],
                            start=(a2 == 0), stop=(a2 == 3), perf_mode=DR)
                t_ = pp.tile([128, N], BF, name=f"z0tT{p}", tag=f"T{p}")
                if p % 2 == 0:
                    nc.vector.tensor_scalar_mul(t_[:], pb[:], 1.0 / 512.0)
                else:
                    nc.scalar.activation(t_[:], pb[:], AF.Copy, scale=1.0 / 512.0)
                Z0T.append(t_)

            # ---- S9: Z0N = 64*transpose(Z0T), fp8 (m-blocks on cols) ----
            z0nb = pp.tile([128, NCH * D], F8, tag="z0nb")
            z0nbv = z0nb[:].rearrange("q (m d) -> q m d", m=NCH)
            for i in range(NCH):
                for grp in range(2):
                    pz = pst.tile([128, 384], BF, tag="tr")
                    for k in range(3):
                        p = grp * 3 + k
                        nc.tensor.transpose(pz[:, k * 128:(k + 1) * 128],
                                            Z0T[p][:, i * 128:(i + 1) * 128],
                                            identb[:])
                    scaled_rr(z0nb[:, i * D + grp * 384:i * D + (grp + 1) * 384],
                              pz[:], 64.0)

            # ---- S10: Z1T[d',n] = sum_m Z0[m,d'] L^T[m,n] = (L@Z0)^T ----
            Z1T = []
            for p in range(DCH):
                pb = psb.tile([128, N], FP, tag="big")
                for h in range(2):
                    for a2 in range(4):
                        nc.tensor.matmul(
                            pb[:, h * 512:(h + 1) * 512],
                            z0nbv[:, 2 * a2:2 * a2 + 2, p * 128:(p + 1) * 128],
                            lt8v[:, 2 * a2:2 * a2 + 2, h * 512:(h + 1) * 512],
                            start=(a2 == 0), stop=(a2 == 3), perf_mode=DR)
                t_ = pp.tile([128, N], BF, name=f"z1tV{p}", tag=f"V{p}")
                scaled_rr(t_[:], pb[:], 1.0 / 512.0)
                Z1T.append(t_)

            # ---- S11: Z1N = 64*transpose(Z1T), fp8 ----
            z1nb = pp.tile([128, NCH * D], F8, tag="z1nb")
            z1nbv = z1nb[:].rearrange("q (m d) -> q m d", m=NCH)
            for i in range(NCH):
                for grp in range(2):
                    pz = pst.tile([128, 384], BF, tag="tr")
                    for k in range(3):
                        p = grp * 3 + k
                        nc.tensor.transpose(pz[:, k * 128:(k + 1) * 128],
                                            Z1T[p][:, i * 128:(i + 1) * 128],
                                            identb[:])
                    scaled_rr(z1nb[:, i * D + grp * 384:i * D + (grp + 1) * 384],
                              pz[:], 64.0)

            # ---- S12: Z2T = 2*(L@Z1)^T - Z0T ----
            Z2T = []
            for p in range(DCH):
                pb = psb.tile([128, N], FP, tag="big")
                for h in range(2):
                    for a2 in range(4):
                        nc.tensor.matmul(
                            pb[:, h * 512:(h + 1) * 512],
                            z1nbv[:, 2 * a2:2 * a2 + 2, p * 128:(p + 1) * 128],
                            lt8v[:, 2 * a2:2 * a2 + 2, h * 512:(h + 1) * 512],
                            start=(a2 == 0), stop=(a2 == 3), perf_mode=DR)
                # psum holds 512*(L@Z1); Z2 = psum/256 - Z0
                zc = sp.tile([128, N], BF, tag="z2c", bufs=2)
                scaled_rr(zc[:], pb[:], 1.0 / 256.0)
                t_ = pp.tile([128, N], BF, name=f"z2tZ{p}", tag=f"Z2{p}")
                nc.vector.tensor_tensor(t_[:], zc[:], Z0T[p][:], op=OP.subtract)
                Z2T.append(t_)

            # ---- S13: projection (Cheb k=0..2 + residual), bias, relu ----
            for q in range(QO):
                p = q // 2
                pb = psb.tile([128, N], FP, tag="big")
                rhs4 = (Z0T[p], Z1T[p], Z2T[p], XTT[p])
                for h in range(2):
                    for k in range(4):
                        nc.tensor.matmul(
                            pb[:, h * 512:(h + 1) * 512],
                            wpb[:, (4 * q + k) * 128:(4 * q + k + 1) * 128],
                            rhs4[k][:, h * 512:(h + 1) * 512],
                            start=(k == 0), stop=(k == 3))
                ob = sp.tile([128, N], BF, tag="outbuf", bufs=2)
                if q % 2 == 0:
                    nc.scalar.activation(ob[:], pb[:], AF.Relu, bias=bias128[:])
                else:
                    nc.vector.tensor_scalar(ob[:], pb[:], bias128[:], 0.0,
                                            op0=OP.add, op1=OP.max)
                nc.sync.dma_start(out_d[q * 128:(q + 1) * 128, :], ob[:])

    nc.compile()
    _compiled["nc"] = nc
    return nc


def _host_prep(x, edge_index, edge_weight, Ve, be, Vs, bs, cheb_W, cheb_b, res_W, res_b):
    import ml_dtypes
    BF = ml_dtypes.bfloat16
    row = np.asarray(edge_index[0]).astype(np.int64)
    col = np.asarray(edge_index[1]).astype(np.int64)
    w = np.asarray(edge_weight, np.float64).copy()
    w[row == col] = 0.0
    deg = np.zeros(N, np.float64)
    np.add.at(deg, row, w)
    dis = np.where(deg > 0, 1.0 / np.sqrt(np.where(deg > 0, deg, 1.0)), 0.0)
    norm = -dis[row] * w * dis[col]
    L = np.zeros((N, N), np.float64)
    np.add.at(L, (col, row), norm)
    LT = np.ascontiguousarray(L.T.astype(np.float32))

    cheb_W = np.asarray(cheb_W, np.float32)
    res_W = np.asarray(res_W, np.float32)
    # wpb[p, (4q+k)*128 + c] = blk(q,k)[p, c]; out^T tile q rows (t,g) with
    # t = 2q + c//64, contracting t-major tile p=q//2 rows (t', f)
    wq = np.zeros((QO, 4, 128, 128), np.float32)
    Wlist = [cheb_W[0], cheb_W[1], cheb_W[2], res_W.T]  # each (F, G)
    for q in range(QO):
        off = 0 if q % 2 == 0 else 2
        for b_ in range(2):
            a = b_ + off
            for k in range(4):
                wq[q, k, 32 * a:32 * a + 32, 64 * b_:64 * b_ + 64] = Wlist[k]
    wpb = np.ascontiguousarray(
        wq.transpose(2, 0, 1, 3).reshape(128, QO * 4 * 128)).astype(BF)

    b64 = (np.asarray(cheb_b, np.float32) + np.asarray(res_b, np.float32))
    bias128 = np.concatenate([b64, b64]).reshape(128, 1).astype(np.float32)

    import ml_dtypes as mld
    bigi = np.zeros((128, 384), np.float32)
    bigi[np.arange(128), 128 + np.arange(128)] = 1.0
    return {
        "bigi": bigi.astype(mld.bfloat16),
        "identb": np.eye(128, dtype=np.float32).astype(BF),
        "ident8": np.eye(128, dtype=np.float32).astype(mld.float8_e4m3),
        "vetb": np.ascontiguousarray(np.asarray(Ve, np.float32).T).astype(BF),
        "be": np.ascontiguousarray(np.asarray(be, np.float32)[0]),
        "vst8": np.ascontiguousarray(
            np.asarray(Vs, np.float32).T).astype(mld.float8_e4m3),
        "bst": np.ascontiguousarray(np.asarray(bs, np.float32)[0]).astype(BF),
        "lt8": (8.0 * LT).astype(mld.float8_e4m3),
        "wpb": wpb,
        "bias128": bias128,
    }


TRACE = False
LAST = {}


def kernel(x, edge_index, edge_weight, Ve, be, Vs, bs, cheb_W, cheb_b, res_W, res_b):
    from concourse.bass_utils import run_bass_kernel_spmd
    import ml_dtypes
    BF = ml_dtypes.bfloat16
    F8H = ml_dtypes.float8_e4m3

    x = np.asarray(x, np.float32)
    shared = _host_prep(x, edge_index, edge_weight, Ve, be, Vs, bs,
                        cheb_W, cheb_b, res_W, res_b)
    nc = _build()
    in_maps = []
    for b in range(B):
        m = dict(shared)
        xb = x[b]                                   # (N, F, T)
        xnp = np.zeros((N, F, 32), np.float32)      # col 32f+t, zero padded
        xnp[:, :, :T] = xb
        m["xnp"] = np.ascontiguousarray(xnp.reshape(N, 1024)).astype(F8H)
        xtf = xb.reshape(N, D).T                    # (768, N), d = f*24+t
        m["xt8f"] = np.ascontiguousarray(xtf).astype(F8H)
        m["xtt"] = np.ascontiguousarray(
            xb.transpose(2, 1, 0).reshape(D, N)).astype(BF)  # d' = t*32+f
        in_maps.append(m)
    res = run_bass_kernel_spmd(nc, in_maps, list(range(B)), trace=TRACE)
    LAST["res"] = res
    out = np.stack(
        [r["out"].astype(np.float32).reshape(T, G, N).transpose(2, 1, 0)
         for r in res.results], axis=0)
    return out
